# revision 1
# baseline (speedup 1.0000x reference)
"""BranchLinear (MoE routing) Trainium2 kernel.

Math: out[t] = x[t] @ weight[branch_idx[t]] + bias[branch_idx[t]]
  x: [131072, 512] f32, branch_idx: [131072] int32 in [0,8),
  weight: [8, 512, 512] f32, bias: [8, 512] f32.

Strategy (data-parallel over 8 NeuronCores, T sharded):
  Per core (16384 tokens): tokens are processed grouped by branch so each
  token is multiplied by exactly one 512x512 weight (1x FLOPs, vs 8x for
  the masked approach). The grouping permutation is cheap host-side
  bookkeeping (argsort of the given routing); all FLOPs and all HBM
  traffic (x gather, weight load, out scatter) happen on device:
    - indirect DMA gathers 128 sorted token rows into SBUF
    - PE transposes the tile (via identity matmul) so D is on partitions
    - 4 accumulated fp32 matmuls against the resident branch weight
    - DVE adds the (pre-broadcast) branch bias
    - indirect DMA scatters the 128 result rows to their original slots
  Branch segments are padded to 128-token tiles (pad slots gather/scatter
  with an out-of-bounds index and are skipped via the DMA bounds check),
  and per-branch slot sizes are the max over cores so one SPMD program
  serves all 8 cores.
"""

import numpy as np

P = 128           # SBUF partitions / tile height (tokens per tile)
NCORES = 8

_prog_cache = {}
TRACE = False          # dev knob: capture NTFF profile on the next run
LAST_RESULTS = None    # BassKernelResults of the last traced run


def _split_multiwaits(nc):
    """This container's walrus build allows at most ONE sync wait per
    instruction (2 for EventSemaphore), but Tile emits instructions with
    several waits. Hoist extra waits onto fresh single-wait nops inserted
    just before the instruction on the same engine (identical blocking
    semantics: the engine's sequencer executes both in program order)."""
    import concourse.mybir as mybir

    uid = 0
    for f in nc.m.functions:
        for bb in f.blocks:
            insts = bb.instructions
            out, changed = [], False
            for ins in insts:
                si = ins.sync_info
                cap = 2 if ins.opcode == "EventSemaphore" else 1
                if si is not None and len(si.on_wait) > cap:
                    waits = list(si.on_wait)
                    for w in waits[cap:]:
                        nop = mybir.InstNoOp(
                            name=f"waitsplit_{uid}",
                            engine=ins.engine,
                            bass_nofuse=True,
                            sync_info=mybir.SyncInfo(on_wait=[w], on_update=[]),
                        )
                        uid += 1
                        nc.register_instruction(nop, overwrite=True)
                        out.append(nop)
                    si.on_wait = waits[:cap]
                    ins.sync_info = si
                    changed = True
                out.append(ins)
            if changed:
                bb.instructions = out


def _build_program(TS, D, NB, branch_of_tile, use_f32r=False, epochs=1):
    """Build the per-core SPMD bass program.

    Inputs (per core): x [TS, D] f32, wr [NB*D, D] f32 (weight reshaped),
    br [1, NB*D] f32 (bias reshaped), gidx/sidx [P, S] int32 (padded sorted
    token ids; gather pads read row 0, scatter pads write the trash rows
    TS..TS+127). Output: out [TS+P, D] f32 (last P rows are trash).
    """
    import concourse.bass as bass
    import concourse.mybir as mybir
    import concourse.tile as tile
    from concourse.masks import make_identity

    f32 = mybir.dt.float32
    mm_dt = mybir.dt.float32r if use_f32r else f32
    KC = D // P                       # contraction chunks (4)
    S = len(branch_of_tile)           # total 128-token tiles

    nc = bass.Bass(name="branch_linear")
    x_d = nc.dram_tensor("x", [TS, D], f32, kind="ExternalInput")
    w_d = nc.dram_tensor("wr", [NB * D, D], f32, kind="ExternalInput")
    b_d = nc.dram_tensor("br", [1, NB * D], f32, kind="ExternalInput")
    gidx_d = nc.dram_tensor("gidx", [P, S], mybir.dt.int32, kind="ExternalInput")
    sidx_d = nc.dram_tensor("sidx", [P, S], mybir.dt.int32, kind="ExternalInput")
    out_d = nc.dram_tensor("out", [TS + P, D], f32, kind="ExternalOutput")

    with tile.TileContext(nc) as tc:
        with (
            tc.tile_pool(name="const", bufs=1) as cpool,
            tc.tile_pool(name="gather", bufs=4) as gpool,
            tc.tile_pool(name="xt", bufs=4) as tpool,
            tc.tile_pool(name="osb", bufs=4) as opool,
            tc.tile_pool(name="ps_t", bufs=2, space="PSUM") as ps_t,
            tc.tile_pool(name="ps_o", bufs=2, space="PSUM") as ps_o,
            tc.tile_pool(name="ps_b", bufs=1, space="PSUM") as ps_b,
        ):
            ident = cpool.tile([P, P], f32, tag="ident")
            make_identity(nc, ident[:])

            gidx_sb = cpool.tile([P, S], mybir.dt.int32, tag="gidx")
            nc.sync.dma_start(gidx_sb[:], gidx_d[:, :])
            sidx_sb = cpool.tile([P, S], mybir.dt.int32, tag="sidx")
            nc.sync.dma_start(sidx_sb[:], sidx_d[:, :])

            # resident weights: one [P, D] SBUF tile per (branch, k-chunk)
            w_sb = {}
            for n in range(NB):
                for k in range(KC):
                    w = cpool.tile([P, D], mm_dt, tag=f"w_{n}_{k}")
                    r0 = (n * KC + k) * P
                    if use_f32r:
                        wstage = gpool.tile([P, D], f32, tag="wstage")
                        nc.sync.dma_start(wstage[:], w_d[r0:r0 + P, :])
                        nc.vector.tensor_copy(out=w[:], in_=wstage[:])
                    else:
                        nc.sync.dma_start(w[:], w_d[r0:r0 + P, :])
                    w_sb[(n, k)] = w

            # bias, broadcast to 128 partitions via K=1 matmul with ones
            bias1p = cpool.tile([1, NB * D], f32, tag="bias1p")
            nc.sync.dma_start(bias1p[:], b_d[:, :])
            ones1p = cpool.tile([1, P], f32, tag="ones1p")
            nc.vector.memset(ones1p[:], 1.0)
            bias_bc = cpool.tile([P, NB * D], f32, tag="bias_bc")
            for n in range(NB):
                pb = ps_b.tile([P, D], f32)
                nc.tensor.matmul(
                    out=pb[:], lhsT=ones1p[:], rhs=bias1p[:, n * D:(n + 1) * D],
                    start=True, stop=True,
                )
                nc.scalar.copy(out=bias_bc[:, n * D:(n + 1) * D], in_=pb[:])

            for s in list(range(S)) * epochs:
                n = branch_of_tile[s]
                # 1) gather 128 sorted token rows (pads skipped)
                xg = gpool.tile([P, D], f32, tag="xg")
                nc.gpsimd.indirect_dma_start(
                    out=xg[:], out_offset=None,
                    in_=x_d[:, :],
                    in_offset=bass.IndirectOffsetOnAxis(
                        ap=gidx_sb[:, s:s + 1], axis=0),
                )
                # 2) transpose tile so D is on partitions (PE, via identity)
                xt_ps = ps_t.tile([P, D], f32)
                for k in range(KC):
                    nc.tensor.transpose(
                        out=xt_ps[:, k * P:(k + 1) * P],
                        in_=xg[:, k * P:(k + 1) * P],
                        identity=ident[:],
                    )
                xt = tpool.tile([P, D], mm_dt, tag="xt")
                nc.scalar.copy(out=xt[:], in_=xt_ps[:])
                # 3) out[tok, :] = sum_k xt[:,k].T @ W[n][k]
                o_ps = ps_o.tile([P, D], f32)
                for k in range(KC):
                    nc.tensor.matmul(
                        out=o_ps[:],
                        lhsT=xt[:, k * P:(k + 1) * P],
                        rhs=w_sb[(n, k)][:],
                        start=(k == 0), stop=(k == KC - 1),
                    )
                # 4) + bias (PSUM -> SBUF)
                o_sb = opool.tile([P, D], f32, tag="osb")
                nc.vector.tensor_add(
                    out=o_sb[:], in0=o_ps[:],
                    in1=bias_bc[:, n * D:(n + 1) * D],
                )
                # 5) scatter rows back to original token slots (pads skipped)
                nc.gpsimd.indirect_dma_start(
                    out=out_d[:, :],
                    out_offset=bass.IndirectOffsetOnAxis(
                        ap=sidx_sb[:, s:s + 1], axis=0),
                    in_=o_sb[:], in_offset=None,
                )
    _split_multiwaits(nc)
    return nc


def _routing(branch_idx, TS, NB):
    """Per-core padded, branch-sorted gather/scatter index arrays.

    Returns (gidx [NCORES][P, S], sidx [NCORES][P, S], branch_of_tile [S]).
    Gather pads read row 0; scatter pads write trash rows TS + partition."""
    ncores = branch_idx.shape[0] // TS
    perms, counts = [], np.zeros((ncores, NB), np.int64)
    for c in range(ncores):
        bi = branch_idx[c * TS:(c + 1) * TS]
        perms.append(np.argsort(bi, kind="stable"))
        counts[c] = np.bincount(bi, minlength=NB)
    slot_tiles = [int(-(-counts[:, n].max() // P)) for n in range(NB)]
    branch_of_tile = []
    for n in range(NB):
        branch_of_tile += [n] * slot_tiles[n]
    S = len(branch_of_tile)

    gidx_arrays, sidx_arrays = [], []
    for c in range(ncores):
        flat = np.full(S * P, -1, np.int64)
        off = base = 0
        for n in range(NB):
            cnt = int(counts[c, n])
            flat[base:base + cnt] = perms[c][off:off + cnt]
            off += cnt
            base += slot_tiles[n] * P
        pad = flat < 0
        gflat = np.where(pad, 0, flat)
        sflat = np.where(pad, TS + (np.arange(S * P) % P), flat)
        gidx_arrays.append(
            np.ascontiguousarray(gflat.reshape(S, P).T.astype(np.int32)))
        sidx_arrays.append(
            np.ascontiguousarray(sflat.reshape(S, P).T.astype(np.int32)))
    return gidx_arrays, sidx_arrays, branch_of_tile


def kernel(x, branch_idx, weight, bias):
    from concourse.bass_utils import run_bass_kernel_spmd

    x = np.ascontiguousarray(np.asarray(x, np.float32))
    branch_idx = np.asarray(branch_idx, np.int32)
    weight = np.ascontiguousarray(np.asarray(weight, np.float32))
    bias = np.ascontiguousarray(np.asarray(bias, np.float32))

    T, D = x.shape
    NB = weight.shape[0]
    TS = T // NCORES

    gidx_arrays, sidx_arrays, branch_of_tile = _routing(branch_idx, TS, NB)

    key = (TS, D, NB, tuple(branch_of_tile))
    if key not in _prog_cache:
        _prog_cache[key] = _build_program(TS, D, NB, branch_of_tile)
    nc = _prog_cache[key]

    wr = np.ascontiguousarray(weight.reshape(NB * D, D))
    br = np.ascontiguousarray(bias.reshape(1, NB * D))
    in_maps = [
        {"x": x[c * TS:(c + 1) * TS], "wr": wr, "br": br,
         "gidx": gidx_arrays[c], "sidx": sidx_arrays[c]}
        for c in range(NCORES)
    ]
    kwargs = {}
    if TRACE:
        kwargs = dict(trace=True)
    res = run_bass_kernel_spmd(nc, in_maps, core_ids=list(range(NCORES)), **kwargs)
    if TRACE:
        global LAST_RESULTS
        LAST_RESULTS = res
    out = np.concatenate(
        [res.results[c]["out"][:TS] for c in range(NCORES)], axis=0)
    return out



# revision 5
# speedup vs baseline: 350.8419x; 350.8419x over previous
"""BranchLinear (MoE routing) Trainium2 kernel.

Math: out[t] = x[t] @ weight[branch_idx[t]] + bias[branch_idx[t]]
  x: [131072, 512] f32, branch_idx: [131072] int32 in [0,8),
  weight: [8, 512, 512] f32, bias: [8, 512] f32.

Strategy (data-parallel over 8 NeuronCores, T sharded, 16384 tokens/core):
  Tokens are processed grouped by branch so each token is multiplied by
  exactly one 512x512 weight (1x FLOPs). The grouping permutation is cheap
  host-side bookkeeping (argsort of the given routing); all FLOPs and all
  HBM traffic (x gather, weight load, out scatter) happen on device:
    - gpsimd dma_gather pulls G*128 branch-sorted token rows per
      instruction into SBUF (one descriptor per row, 16 SDMA engines,
      ~258 GB/s measured)
    - PE transposes each 128-token tile via identity matmul so D is on
      partitions; ACT copies PSUM->SBUF
    - 4 accumulated matmuls against the resident branch weight in
      float32r (1 cyc/row; plain fp32 needs 2 half-rate passes = 4x)
    - DVE adds the (pre-broadcast) branch bias
    - gpsimd dma_scatter_add writes G*128 rows back to their original
      token slots. The output is split into Q=2 row-range tensors and
      groups alternate ranges so consecutive scatters hit different
      tensors - otherwise Tile's WAW tracking serializes them on the
      ~2us HBM-write completion latency (measured 2x throughput).
  Tokens are sorted by (branch, output-range); each (branch, range)
  segment is padded to 128-token tiles (pads gather row 0 and scatter to
  per-range trash rows R..R+127), per-(branch,range) slot sizes are the
  max over cores so one SPMD program serves all 8 cores, and each range's
  tile list is padded to a multiple of G so gather/scatter groups are
  range-pure.
"""

import numpy as np

P = 128           # SBUF partitions / tile height (tokens per tile)
NCORES = 8
G = 8             # tiles per dma_gather / dma_scatter_add group
Q = 2             # output row-range splits (independent scatter WAW chains)

_prog_cache = {}


def _split_multiwaits(nc):
    """This container's walrus build allows at most ONE sync wait per
    instruction (2 for EventSemaphore), but Tile emits instructions with
    several waits. Hoist extra waits onto fresh single-wait nops inserted
    just before the instruction on the same engine (identical blocking
    semantics: the engine's sequencer executes both in program order)."""
    import concourse.mybir as mybir

    uid = 0
    for f in nc.m.functions:
        for bb in f.blocks:
            insts = bb.instructions
            out, changed = [], False
            for ins in insts:
                si = ins.sync_info
                cap = 2 if ins.opcode == "EventSemaphore" else 1
                if si is not None and len(si.on_wait) > cap:
                    waits = list(si.on_wait)
                    for w in waits[cap:]:
                        nop = mybir.InstNoOp(
                            name=f"waitsplit_{uid}",
                            engine=ins.engine,
                            bass_nofuse=True,
                            sync_info=mybir.SyncInfo(on_wait=[w], on_update=[]),
                        )
                        uid += 1
                        nc.register_instruction(nop, overwrite=True)
                        out.append(nop)
                    si.on_wait = waits[:cap]
                    ins.sync_info = si
                    changed = True
                out.append(ins)
            if changed:
                bb.instructions = out
    return nc


def _build_program(TS, D, NB, groups, epochs=1):
    """Build the per-core SPMD bass program.

    groups: list of (q, (branch,)*G) - gather/scatter group schedule.
    Inputs (per core): x [TS, D] f32r, wr [NB*D, D] f32r, br [1, NB*D] f32,
    gidx/sidx [128, NGtot*G*P/16] int16 (dma_gather wrapping: group g's
    column block holds flat slots i at (i%16 + 16*replica, i//16); gather
    pads read row 0, scatter pads hit local trash rows R..R+P-1).
    Outputs: out0/out1 [R + P, D] f32 (R = TS//Q; last P rows are trash).
    """
    import concourse.bass as bass
    import concourse.mybir as mybir
    import concourse.tile as tile
    from concourse import library_config
    from concourse.masks import make_identity

    f32 = mybir.dt.float32
    f32r = mybir.dt.float32r
    KC = D // P                       # contraction chunks (4)
    NGT = len(groups)
    NIDX = G * P                      # indices per group
    COLS = NIDX // 16                 # int16 idx columns per group
    R = TS // Q

    nc = bass.Bass(name="branch_linear")
    # x and wr are declared float32r (same 32-bit layout as f32) so the PE
    # streams them at full rate; the host feeds plain float32 arrays.
    x_d = nc.dram_tensor("x", [TS, D], f32r, kind="ExternalInput")
    w_d = nc.dram_tensor("wr", [NB * D, D], f32r, kind="ExternalInput")
    b_d = nc.dram_tensor("br", [1, NB * D], f32, kind="ExternalInput")
    gidx_d = nc.dram_tensor("gidx", [P, NGT * COLS], mybir.dt.int16,
                            kind="ExternalInput")
    sidx_d = nc.dram_tensor("sidx", [P, NGT * COLS], mybir.dt.int16,
                            kind="ExternalInput")
    out_d = [nc.dram_tensor(f"out{q}", [R + P, D], f32, kind="ExternalOutput")
             for q in range(Q)]

    with tile.TileContext(nc) as tc:
        with (
            tc.tile_pool(name="const", bufs=1) as cpool,
            tc.tile_pool(name="gather", bufs=3) as gpool,
            tc.tile_pool(name="xt", bufs=4) as tpool,
            tc.tile_pool(name="osb", bufs=3) as opool,
            tc.tile_pool(name="ps_t", bufs=2, space="PSUM") as ps_t,
            tc.tile_pool(name="ps_o", bufs=2, space="PSUM") as ps_o,
            tc.tile_pool(name="ps_b", bufs=1, space="PSUM") as ps_b,
        ):
            nc.gpsimd.load_library(library_config.mlp)
            nreg = nc.gpsimd.to_reg(NIDX)

            ident_f32 = cpool.tile([P, P], f32, tag="ident_f32")
            make_identity(nc, ident_f32[:])
            ident = cpool.tile([P, P], f32r, tag="ident")
            nc.vector.tensor_copy(out=ident[:], in_=ident_f32[:])

            gidx_sb = cpool.tile([P, NGT * COLS], mybir.dt.int16, tag="gidx")
            nc.sync.dma_start(gidx_sb[:], gidx_d[:, :])
            sidx_sb = cpool.tile([P, NGT * COLS], mybir.dt.int16, tag="sidx")
            nc.sync.dma_start(sidx_sb[:], sidx_d[:, :])

            # resident weights: one [P, D] SBUF tile per (branch, k-chunk)
            w_sb = {}
            for n in range(NB):
                for k in range(KC):
                    w = cpool.tile([P, D], f32r, tag=f"w_{n}_{k}")
                    r0 = (n * KC + k) * P
                    nc.sync.dma_start(w[:], w_d[r0:r0 + P, :])
                    w_sb[(n, k)] = w

            # bias, broadcast to 128 partitions via K=1 matmul with ones
            bias1p = cpool.tile([1, NB * D], f32, tag="bias1p")
            nc.sync.dma_start(bias1p[:], b_d[:, :])
            ones1p = cpool.tile([1, P], f32, tag="ones1p")
            nc.vector.memset(ones1p[:], 1.0)
            bias_bc = cpool.tile([P, NB * D], f32, tag="bias_bc")
            for n in range(NB):
                pb = ps_b.tile([P, D], f32)
                nc.tensor.matmul(
                    out=pb[:], lhsT=ones1p[:], rhs=bias1p[:, n * D:(n + 1) * D],
                    start=True, stop=True,
                )
                nc.scalar.copy(out=bias_bc[:, n * D:(n + 1) * D], in_=pb[:])

            for g in list(range(NGT)) * epochs:
                q, branches = groups[g]
                cols = slice(g * COLS, (g + 1) * COLS)
                # 1) gather G*128 sorted token rows (pads read row 0)
                xg = gpool.tile([P, G, D], f32r, tag="xg")
                nc.gpsimd.dma_gather(
                    out_ap=xg[:, :, :], in_ap=x_d[:, :],
                    idxs_ap=gidx_sb[:, cols],
                    num_idxs=NIDX, num_idxs_reg=nreg, elem_size=D)
                ob = opool.tile([P, G, D], f32, tag="ob")
                for j in range(G):
                    n = branches[j]
                    # 2) transpose tile so D is on partitions (PE, identity)
                    xt_ps = ps_t.tile([P, D], f32r)
                    for k in range(KC):
                        nc.tensor.transpose(
                            out=xt_ps[:, k * P:(k + 1) * P],
                            in_=xg[:, j, k * P:(k + 1) * P],
                            identity=ident[:],
                        )
                    xt = tpool.tile([P, D], f32r, tag="xt")
                    nc.scalar.copy(out=xt[:], in_=xt_ps[:])
                    # 3) out[tok, :] = sum_k xt[:,k].T @ W[n][k]  (f32r)
                    o_ps = ps_o.tile([P, D], f32)
                    for k in range(KC):
                        nc.tensor.matmul(
                            out=o_ps[:],
                            lhsT=xt[:, k * P:(k + 1) * P],
                            rhs=w_sb[(n, k)][:],
                            start=(k == 0), stop=(k == KC - 1),
                        )
                    # 4) + bias (PSUM -> SBUF)
                    nc.vector.tensor_add(
                        out=ob[:, j, :], in0=o_ps[:],
                        in1=bias_bc[:, n * D:(n + 1) * D],
                    )
                # 5) scatter-add G*128 rows to range-q slots (out zero-init)
                nc.gpsimd.dma_scatter_add(
                    out_ap=out_d[q][:, :], in_ap=ob[:, :, :],
                    idxs_ap=sidx_sb[:, cols],
                    num_idxs=NIDX, num_idxs_reg=nreg, elem_size=D)
    _split_multiwaits(nc)
    mybir.codegen_inst_isa_subclasses(nc)
    return nc


def _wrap16(flat):
    """flat [n] -> [128, n/16] int16: flat[i] at (i%16, i//16), the 16-row
    block replicated 8x across partitions (one copy per Q7 core)."""
    n = len(flat)
    blk = np.asarray(flat, np.int16).reshape(n // 16, 16).T
    return np.tile(blk, (8, 1))


def _routing(branch_idx, TS, NB):
    """Per-core branch+range-sorted gather/scatter index arrays.

    Returns (gidx [NCORES][128, NGT*G*P/16] int16,
             sidx [NCORES][128, NGT*G*P/16] int16,
             groups [(q, (branch,)*G)]).
    Gather pads read row 0; scatter pads write local trash rows R..R+P-1."""
    ncores = branch_idx.shape[0] // TS
    R = TS // Q
    counts = np.zeros((ncores, NB, Q), np.int64)
    perms = []
    for c in range(ncores):
        bi = branch_idx[c * TS:(c + 1) * TS].astype(np.int64)
        rq = np.arange(TS, dtype=np.int64) // R
        key = bi * Q + rq
        perms.append(np.argsort(key, kind="stable"))
        for n in range(NB):
            for q in range(Q):
                counts[c, n, q] = int(((bi == n) & (rq == q)).sum())

    # (branch, range) slot sizes: max over cores, padded to whole tiles
    slot_tiles = np.zeros((NB, Q), np.int64)
    for n in range(NB):
        for q in range(Q):
            slot_tiles[n, q] = -(-counts[:, n, q].max() // P)

    # tile schedule: per range, branch-major tile list padded to G multiple;
    # groups alternate ranges so consecutive scatters hit different tensors
    range_tiles = {q: [] for q in range(Q)}  # list of (branch | None)
    for q in range(Q):
        for n in range(NB):
            range_tiles[q] += [n] * int(slot_tiles[n, q])
        while len(range_tiles[q]) % G:
            range_tiles[q].append(0)  # all-pad tile
    range_groups = {q: [tuple(range_tiles[q][i * G:(i + 1) * G])
                        for i in range(len(range_tiles[q]) // G)]
                    for q in range(Q)}
    groups = []
    gi = {q: 0 for q in range(Q)}
    while any(gi[q] < len(range_groups[q]) for q in range(Q)):
        for q in range(Q):
            if gi[q] < len(range_groups[q]):
                groups.append((q, range_groups[q][gi[q]]))
                gi[q] += 1

    # slot base offset of each (branch, range) segment in the sorted order
    # and in the padded tile schedule
    gidx_arrays, sidx_arrays = [], []
    # tile start index within its range's tile list, per (n, q)
    tile_base = np.zeros((NB, Q), np.int64)
    for q in range(Q):
        b = 0
        for n in range(NB):
            tile_base[n, q] = b
            b += slot_tiles[n, q]
    # map: for each group (issue order), its tiles' positions in range list
    for c in range(ncores):
        # per (n, q): the sorted token ids of this core
        bi = branch_idx[c * TS:(c + 1) * TS].astype(np.int64)
        perm = perms[c]
        # fill per-range padded slot arrays
        slots = {q: np.full(len(range_tiles[q]) * P, -1, np.int64)
                 for q in range(Q)}
        off = 0
        for n in range(NB):
            for q in range(Q):
                cnt = int(counts[c, n, q])
                base = int(tile_base[n, q]) * P
                slots[q][base:base + cnt] = perm[off:off + cnt]
                off += cnt
        gflat_groups, sflat_groups = [], []
        gi = {q: 0 for q in range(Q)}
        for q, _branches in groups:
            i0 = gi[q] * G * P
            seg = slots[q][i0:i0 + G * P]
            gi[q] += 1
            pad = seg < 0
            gseg = np.where(pad, 0, seg)
            # local output row within range q; pads -> trash rows R..R+P-1
            sseg = np.where(pad, R + (np.arange(G * P) % P), seg - q * R)
            gflat_groups.append(gseg)
            sflat_groups.append(sseg)
        gidx_arrays.append(np.ascontiguousarray(
            np.concatenate([_wrap16(s) for s in gflat_groups], axis=1)))
        sidx_arrays.append(np.ascontiguousarray(
            np.concatenate([_wrap16(s) for s in sflat_groups], axis=1)))
    return gidx_arrays, sidx_arrays, groups


def kernel(x, branch_idx, weight, bias):
    from concourse.bass_utils import run_bass_kernel_spmd

    x = np.ascontiguousarray(np.asarray(x, np.float32))
    branch_idx = np.asarray(branch_idx, np.int32)
    weight = np.ascontiguousarray(np.asarray(weight, np.float32))
    bias = np.ascontiguousarray(np.asarray(bias, np.float32))

    T, D = x.shape
    NB = weight.shape[0]
    TS = T // NCORES
    R = TS // Q

    gidx_arrays, sidx_arrays, groups = _routing(branch_idx, TS, NB)

    key = (TS, D, NB, tuple(groups))
    if key not in _prog_cache:
        _prog_cache[key] = _build_program(TS, D, NB, groups)
    nc = _prog_cache[key]

    wr = np.ascontiguousarray(weight.reshape(NB * D, D))
    br = np.ascontiguousarray(bias.reshape(1, NB * D))
    in_maps = [
        {"x": x[c * TS:(c + 1) * TS], "wr": wr, "br": br,
         "gidx": gidx_arrays[c], "sidx": sidx_arrays[c]}
        for c in range(NCORES)
    ]
    res = run_bass_kernel_spmd(nc, in_maps, core_ids=list(range(NCORES)))
    out = np.concatenate(
        [np.concatenate([res.results[c][f"out{q}"][:R] for q in range(Q)],
                        axis=0)
         for c in range(NCORES)], axis=0)
    return out
